# revision 2
# baseline (speedup 1.0000x reference)
"""Trainium2 Bass kernel for nn_DeformableGCN (GNN message passing).

Strategy (1D graph partitioning over 8 NeuronCores):
  - Destination nodes are assigned to cores/tiles via a degree-sorted
    permutation pi: each 128-node tile holds nodes with nearly equal
    in-degree, so each dst node's in-edges occupy its own SBUF partition
    across a minimal number of 128-edge chunks ("identity scatter": the
    segment-sum matmul uses a constant identity weight matrix).
  - Node tables are fp16 with 128-element (256 B) rows [payload | pad].
    All gather descriptors are 256 B (the minimum), matmuls run at the
    1-cycle/row fp16 rate, and table AllGathers move half the bytes of
    the f32 layout for the conv tables.
  - Tables are split POSITIONALLY into two halves: tile positions
    tau < TA form table A, the rest table B (each below 32768 rows, so
    int16 gather indices address them directly).  Each pass issues the
    AllGather of half A as soon as tiles 0..TA-1 are done, overlapping
    it with the rest of the pass; the next pass's A-stream gathers start
    while half B's AllGather is still in flight.  This hides most of the
    collective latency that a monolithic AllGather would expose.
  - Per-edge source rows are fetched with the custom dma_gather
    instruction (int16 indices, 4 SWDGE queues); padding slots gather a
    guaranteed-zero fake-node row.
"""
import os
import sys

sys.path.insert(0, "/opt/trn_rl_repo")

import numpy as np

import concourse.bass as bass
import concourse.bacc as bacc
import concourse.mybir as mybir
import concourse.tile as tile
from concourse.masks import make_identity

M = 8            # cores
P = 128          # partitions
WIN = 64         # gather-call window, in 128-edge chunks
ROW = 128        # fp16 table row width (256 B)
F32 = mybir.dt.float32
F16 = mybir.dt.float16
I16 = mybir.dt.int16
NEG_SLOPE = 0.01


# ------------------------------------------------------------- pjrt runner

class _Runner:
    """Builds the jitted PJRT callable once; repeated exec without retrace."""

    def __init__(self, nc, n_cores):
        import jax
        from jax.sharding import Mesh, PartitionSpec
        from jax.experimental.shard_map import shard_map
        from concourse.bass2jax import (
            install_neuronx_cc_hook, _bass_exec_p, partition_id_tensor)
        install_neuronx_cc_hook()
        self.jax = jax
        self.n_cores = n_cores
        in_names, out_names, out_avals, zero_outs = [], [], [], []
        partition_name = (nc.partition_id_tensor.name
                          if nc.partition_id_tensor else None)
        for alloc in nc.m.functions[0].allocations:
            if not isinstance(alloc, mybir.MemoryLocationSet):
                continue
            name = alloc.memorylocations[0].name
            if alloc.kind == "ExternalInput":
                if name != partition_name:
                    in_names.append(name)
            elif alloc.kind == "ExternalOutput":
                shape = tuple(alloc.tensor_shape)
                dtype = mybir.dt.np(alloc.dtype)
                out_names.append(name)
                out_avals.append(jax.core.ShapedArray(shape, dtype))
                zero_outs.append(np.zeros(shape, dtype))
        self.in_names, self.out_names = in_names, out_names
        self.zero_outs = zero_outs
        n_params = len(in_names)
        all_in_names = list(in_names) + list(out_names)
        if partition_name is not None:
            all_in_names.append(partition_name)

        def _body(*args):
            operands = list(args)
            if partition_name is not None:
                operands.append(partition_id_tensor())
            outs = _bass_exec_p.bind(
                *operands,
                out_avals=tuple(out_avals),
                in_names=tuple(all_in_names),
                out_names=tuple(out_names),
                lowering_input_output_aliases=(),
                sim_require_finite=True,
                sim_require_nnan=True,
                nc=nc,
            )
            return tuple(outs)

        donate = tuple(range(n_params, n_params + len(out_names)))
        devices = jax.devices()[:n_cores]
        self.mesh = Mesh(np.asarray(devices), ("core",))
        in_specs = (PartitionSpec("core"),) * (n_params + len(out_names))
        out_specs = (PartitionSpec("core"),) * len(out_names)
        self.fn = jax.jit(
            shard_map(_body, mesh=self.mesh, in_specs=in_specs,
                      out_specs=out_specs, check_rep=False),
            donate_argnums=donate, keep_unused=True)
        self._dev_inputs = None

    def place_inputs(self, in_maps):
        import jax
        from jax.sharding import PartitionSpec, NamedSharding
        per_core = [[np.asarray(m[n]) for n in self.in_names]
                    for m in in_maps]
        arrs = []
        for i, n in enumerate(self.in_names):
            concat = np.concatenate(
                [per_core[c][i] for c in range(self.n_cores)], axis=0)
            arrs.append(jax.device_put(
                concat, NamedSharding(self.mesh, PartitionSpec("core"))))
        for a in arrs:
            a.block_until_ready()
        self._dev_inputs = arrs

    def _zeros(self):
        return [np.zeros((self.n_cores * z.shape[0], *z.shape[1:]), z.dtype)
                for z in self.zero_outs]

    def exec_async(self):
        return self.fn(*self._dev_inputs, *self._zeros())

    def run(self, in_maps=None):
        if in_maps is not None:
            self.place_inputs(in_maps)
        outs = [np.asarray(o) for o in self.exec_async()]
        res = []
        for c in range(self.n_cores):
            d = {}
            for i, n in enumerate(self.out_names):
                per = outs[i].reshape(
                    (self.n_cores, outs[i].shape[0] // self.n_cores)
                    + outs[i].shape[1:])
                d[n] = per[c]
            res.append(d)
        return res

    def time_exec(self, k=8):
        import time
        o = self.exec_async()
        self.jax.block_until_ready(o)
        t0 = time.perf_counter()
        outs = [self.exec_async() for _ in range(k)]
        self.jax.block_until_ready(outs)
        return (time.perf_counter() - t0) / k


# ---------------------------------------------------------------- schedule

def _build_schedule(src, dst, n_nodes):
    """Host-side graph partitioning: permutation, slots, gather indices."""
    E = src.shape[0]
    NP = -(-n_nodes // (M * P)) * (M * P)
    if NP - n_nodes < 2:
        NP += M * P  # guarantee >=2 fake (zero) nodes for padding
    TPC = NP // (M * P)
    TA = (TPC + 1) // 2
    TB = TPC - TA
    rowsA = M * TA * P
    rowsB = M * TB * P
    assert rowsA < 32768 and rowsB < 32768

    deg = np.bincount(dst, minlength=NP).astype(np.int64)

    # Pin table-half ("A") membership by original id so the degree sort
    # below can use the exact final per-half in-degrees.
    is_A = np.zeros(NP, bool)
    nA_real = min(rowsA - 1, n_nodes)
    is_A[:nA_real] = True
    need = rowsA - nA_real
    assert need >= 1 and NP - n_nodes - need >= 1
    is_A[n_nodes:n_nodes + need] = True
    assert is_A.sum() == rowsA

    eA_n = is_A[src]
    dA = np.bincount(dst[eA_n], minlength=NP)
    dB = deg - dA

    # snake order: -dA primary; within each dA group alternate dB
    # direction so dB varies continuously across group boundaries
    snake_dB = np.where(dA % 2 == 0, dB, -dB)
    keys = np.lexsort((-snake_dB, -dA))
    a_order = keys[is_A[keys]]
    b_order = keys[~is_A[keys]]

    # form tiles (consecutive 128 nodes of each pool), then assign tiles
    # to (core, position) so the 8 tiles at each position have matched
    # per-half max in-degrees (minimizes padding)
    a_tiles = a_order.reshape(-1, P)
    b_tiles = b_order.reshape(-1, P)
    tiles = ([("A", t) for t in a_tiles] + [("B", t) for t in b_tiles])
    ca_t = np.array([dA[t].max() for _, t in tiles])
    cb_t = np.array([dB[t].max() for _, t in tiles])
    quota_A = [list(range(M)) if tau < TA else [] for tau in range(TPC)]
    quota_B = [[] if tau < TA else list(range(M)) for tau in range(TPC)]
    kinds = np.array([0 if k == "A" else 1 for k, _ in tiles])
    a_ids = np.flatnonzero(kinds == 0)
    b_ids = np.flatnonzero(kinds == 1)
    a_sorted = a_ids[np.lexsort((-cb_t[a_ids], -ca_t[a_ids]))]
    b_sorted = b_ids[np.lexsort((-cb_t[b_ids], -ca_t[b_ids]))]
    BAND = 8
    cur_a = np.zeros(TPC, np.int64)
    cur_b = np.zeros(TPC, np.int64)
    assign = np.zeros(len(tiles), np.int64)
    # Positions tau < TA only take A tiles and vice versa, so band the two
    # position ranges separately.
    for kind, sorted_ids, quota in ((0, a_sorted, quota_A),
                                    (1, b_sorted, quota_B)):
        taus_all = [t for t in range(TPC) if quota[t]]
        p = 0
        for b0 in range(0, len(taus_all), BAND):
            taus = taus_all[b0: b0 + BAND]
            rem = {t: len(quota[t]) for t in taus}
            nband = sum(rem.values())
            band = list(sorted_ids[p: p + nband])
            p += nband
            band.sort(key=lambda ti: -cb_t[ti])
            for ti in band:
                best, best_cost = -1, None
                for tau in taus:
                    if rem[tau] == 0:
                        continue
                    d = (max(cur_a[tau], ca_t[ti]) - cur_a[tau]
                         + max(cur_b[tau], cb_t[ti]) - cur_b[tau])
                    cost = (d, rem[tau])
                    if best_cost is None or cost < best_cost:
                        best, best_cost = tau, cost
                tau = best
                rem[tau] -= 1
                cur_a[tau] = max(cur_a[tau], ca_t[ti])
                cur_b[tau] = max(cur_b[tau], cb_t[ti])
                assign[ti] = tau

    # local-search refinement: swap same-kind tiles between positions
    members = [[[] for _ in range(TPC)] for _ in range(2)]
    for ti in range(len(tiles)):
        members[kinds[ti]][assign[ti]].append(ti)

    def pos_cost(t):
        tis = members[0][t] + members[1][t]
        return (max((ca_t[i] for i in tis), default=0)
                + max((cb_t[i] for i in tis), default=0))

    rng_ls = np.random.default_rng(0)
    for _ in range(60000):
        k = int(rng_ls.integers(0, 2))
        p, q = (int(v) for v in rng_ls.integers(0, TPC, 2))
        if p == q or not members[k][p] or not members[k][q]:
            continue
        i = members[k][p][int(rng_ls.integers(len(members[k][p])))]
        j = members[k][q][int(rng_ls.integers(len(members[k][q])))]
        before = pos_cost(p) + pos_cost(q)
        members[k][p].remove(i)
        members[k][q].remove(j)
        members[k][p].append(j)
        members[k][q].append(i)
        if pos_cost(p) + pos_cost(q) >= before:
            members[k][p].remove(j)
            members[k][q].remove(i)
            members[k][p].append(i)
            members[k][q].append(j)

    pi = np.empty(NP, np.int64)
    for k, quota in ((0, quota_A), (1, quota_B)):
        for tau in range(TPC):
            for ti in members[k][tau]:
                c = quota[tau].pop()
                pi[tiles[ti][1]] = (c * TPC + tau) * P + np.arange(P)
    assert all(not q for q in quota_A) and all(not q for q in quota_B)

    # table-row addressing: position (c, tau, j) -> half + row
    c_of = pi // (TPC * P)
    tau_of = (pi % (TPC * P)) // P
    j_of = pi % P
    in_A_pos = tau_of < TA
    assert np.array_equal(in_A_pos, is_A)
    tab_row = np.where(
        in_A_pos,
        (c_of * TA + tau_of) * P + j_of,
        (c_of * TB + (tau_of - TA)) * P + j_of)

    pi_dst = pi[dst]
    eA = eA_n

    fakes = np.arange(n_nodes, NP)
    a_fakes = fakes[is_A[fakes]]
    b_fakes = fakes[~is_A[fakes]]
    A_PAD = int(tab_row[a_fakes[0]])
    B_PAD = int(tab_row[b_fakes[0]])

    nA_cnt = np.bincount(pi_dst[eA], minlength=NP)
    nB_cnt = np.bincount(pi_dst[~eA], minlength=NP)
    CA = nA_cnt.reshape(M, TPC, P).max(axis=2).max(axis=0)
    CB = nB_cnt.reshape(M, TPC, P).max(axis=2).max(axis=0)
    base = np.zeros(TPC + 1, np.int64)
    base[1:] = np.cumsum(CA + CB)
    TOTC = int(base[-1])

    key = pi_dst * 2 + (~eA).astype(np.int64)
    eorder = np.argsort(key, kind="stable")
    ks = key[eorder]
    new_grp = np.ones(E, bool)
    new_grp[1:] = ks[1:] != ks[:-1]
    starts = np.flatnonzero(new_grp)
    grp_id = np.cumsum(new_grp) - 1
    rank_in_grp = np.arange(E) - starts[grp_id]
    tau_e = (pi_dst[eorder] % (TPC * P)) // P
    kchunk = rank_in_grp + np.where(ks % 2 == 0, 0, CA[tau_e])
    core_e = pi_dst[eorder] // (TPC * P)
    j_e = pi_dst[eorder] % P
    slot = (base[tau_e] + kchunk) * P + j_e

    chunk_is_A = np.zeros(TOTC, bool)
    for t in range(TPC):
        chunk_is_A[base[t]: base[t] + CA[t]] = True

    idx_flat = np.where(chunk_is_A[None, :, None], np.int16(A_PAD),
                        np.int16(B_PAD)).astype(np.int16)
    idx_flat = np.broadcast_to(idx_flat, (M, TOTC, P)).reshape(M, TOTC * P)
    idx_flat = np.ascontiguousarray(idx_flat)
    vals = tab_row[src[eorder]]
    idx_flat[core_e, slot] = vals.astype(np.int16)

    a_cids = np.flatnonzero(chunk_is_A)
    b_cids = np.flatnonzero(~chunk_is_A)
    streams = {"A": a_cids, "B": b_cids}
    windows = []
    chunk_loc = {}
    col16 = 0
    for sname in ("A", "B"):
        cids = streams[sname]
        for wi0 in range(0, len(cids), WIN):
            wcids = cids[wi0: wi0 + WIN]
            swi = wi0 // WIN
            windows.append((sname, swi, len(wcids), col16))
            for sslot, cid in enumerate(wcids):
                chunk_loc[int(cid)] = (sname, swi, sslot)
            col16 += len(wcids) * P // 16
    TOT16 = col16

    idx_res = np.zeros((M, 128, TOT16), np.int16)
    for c in range(M):
        for (sname, swi, nch, off) in windows:
            cids = streams[sname][swi * WIN: swi * WIN + nch]
            block = idx_flat[c].reshape(TOTC, P)[cids].reshape(-1)
            wr = block.reshape(-1, 16).T
            idx_res[c, :, off: off + nch * P // 16] = np.tile(wr, (8, 1))

    rdeg_pi = np.empty(NP, np.float32)
    rdeg_pi[pi] = (1.0 / np.maximum(deg, 1.0)).astype(np.float32)
    rdeg_ct = rdeg_pi.reshape(M, TPC, P).transpose(0, 2, 1)

    return dict(
        E=E, NP=NP, TPC=TPC, TA=TA, TB=TB, rowsA=rowsA, rowsB=rowsB,
        TOTC=TOTC, TOT16=TOT16,
        pi=pi, tab_row=tab_row, is_A=is_A, CA=CA, CB=CB, base=base,
        windows=windows, chunk_loc=chunk_loc,
        streams=streams, idx_res=idx_res,
        rdeg_ct=np.ascontiguousarray(rdeg_ct),
    )


# ---------------------------------------------------------------- program

def _build_program(s, D, DH, DO, repeat=1):
    NP, TPC, TOT16 = s["NP"], s["TPC"], s["TOT16"]
    TA, TB = s["TA"], s["TB"]
    rowsA, rowsB = s["rowsA"], s["rowsB"]
    CA, CB, base = s["CA"], s["CB"], s["base"]
    windows, chunk_loc = s["windows"], s["chunk_loc"]
    NSH = TPC * P

    nc = bacc.Bacc("TRN2", target_bir_lowering=False, debug=False,
                   enable_asserts=False, num_devices=M, num_swdge_queues=4)

    xtabA_in = nc.dram_tensor("xtabA_in", [rowsA, ROW], F16,
                              kind="ExternalInput")
    xtabB_in = nc.dram_tensor("xtabB_in", [rowsB, ROW], F16,
                              kind="ExternalInput")
    x_shard_t = nc.dram_tensor("x_shard_t", [P, TPC * D], F32,
                               kind="ExternalInput")
    idx_in = nc.dram_tensor("idx_in", [P, TOT16], I16, kind="ExternalInput")
    rdeg_in = nc.dram_tensor("rdeg_in", [P, TPC], F32, kind="ExternalInput")
    wcat1_in = nc.dram_tensor("wcat1_in", [D, D + 2], F32, kind="ExternalInput")
    wcat2_in = nc.dram_tensor("wcat2_in", [DH, DO + 2], F32,
                              kind="ExternalInput")
    params_in = nc.dram_tensor("params_in", [P, 2], F32, kind="ExternalInput")
    out_sh = nc.dram_tensor("out_sh", [NSH, DO], F32, kind="ExternalOutput")

    RG = [list(range(M))]

    with tile.TileContext(nc) as tc:
        with (
            tc.tile_pool(name="consts", bufs=1) as cp,
            tc.tile_pool(name="glo", bufs=3) as glop,
            tc.tile_pool(name="ghi", bufs=3) as ghip,
            tc.tile_pool(name="work", bufs=3) as wp,
            tc.tile_pool(name="small", bufs=4) as sp,
            tc.tile_pool(name="fpsum", bufs=3, space="PSUM") as fpp,
            tc.tile_pool(name="tpsum", bufs=2, space="PSUM") as tpp,
            tc.tile_pool(name="mpsum", bufs=2, space="PSUM") as mpp,
            tc.tile_pool(name="dram", bufs=1, space="DRAM") as dp,
        ):
            ident = cp.tile([P, P], F16, name="ident")
            make_identity(nc, ident[:])
            idxt = cp.tile([P, TOT16], I16, name="idxt")
            nc.sync.dma_start(out=idxt[:], in_=idx_in[:])
            rdeg = cp.tile([P, TPC], F32, name="rdeg")
            nc.sync.dma_start(out=rdeg[:], in_=rdeg_in[:])
            wcat1 = cp.tile([D, D + 2], F32, name="wcat1")
            nc.sync.dma_start(out=wcat1[:], in_=wcat1_in[:])
            wcat2 = cp.tile([DH, DO + 2], F32, name="wcat2")
            nc.sync.dma_start(out=wcat2[:], in_=wcat2_in[:])
            params = cp.tile([P, 2], F32, name="params")
            nc.sync.dma_start(out=params[:], in_=params_in[:])
            acc = cp.tile([P, TPC * D], F32, name="acc")
            adst1 = cp.tile([P, TPC], F32, name="adst1")
            adst2 = cp.tile([P, TPC], F32, name="adst2")

            def emit_gathers(tabA_ap, tabB_ap, tag):
                bufs = {}
                qn = 0
                for (sname, swi, nch, off) in windows:
                    pool = glop if sname == "A" else ghip
                    b = pool.tile([P, WIN * ROW], F16,
                                  name=f"g{tag}{sname}{swi}", tag=f"g{sname}")
                    num = nch * P
                    src_ap = tabA_ap if sname == "A" else tabB_ap
                    nc.gpsimd.dma_gather(
                        out_ap=b[:, : nch * ROW].rearrange(
                            "p (c d) -> p c d", d=ROW),
                        in_ap=src_ap,
                        idxs_ap=idxt[:, off: off + nch * P // 16],
                        num_idxs=num,
                        num_idxs_reg=num,
                        elem_size=ROW,
                        single_packet=False,
                        queue_num=qn % 4,
                    )
                    qn += 1
                    bufs[(sname, swi)] = b
                return bufs

            def chunk_groups(t):
                runs = []
                for cid in range(int(base[t]), int(base[t + 1])):
                    sname, swi, sslot = chunk_loc[cid]
                    if runs and runs[-1][0] == (sname, swi) and \
                            runs[-1][1] + runs[-1][2] == sslot:
                        runs[-1] = (runs[-1][0], runs[-1][1], runs[-1][2] + 1)
                    else:
                        runs.append(((sname, swi), sslot, 1))
                return runs

            def proj_tile(t, xt_ap, wcat_t, din, dout, dest_a, dest_b,
                          adst_sb, bcol, tag):
                """rows [X@W | u] (fp16); saves a_dst column (+bias)."""
                tp = tpp.tile([din, P], F32, name=f"tp{tag}_{t}", tag="tps")
                nc.tensor.transpose(out=tp[:], in_=xt_ap, identity=identf[:])
                xT = sp.tile([din, P], F32, name=f"xT{tag}_{t}", tag="xT")
                nc.scalar.activation(out=xT[:], in_=tp[:],
                                     func=mybir.ActivationFunctionType.Copy)
                mp = mpp.tile([P, dout + 2], F32, name=f"mp{tag}_{t}",
                              tag="mps")
                nc.tensor.matmul(out=mp[:], lhsT=xT[:], rhs=wcat_t[:],
                                 start=True, stop=True)
                row = wp.tile([P, ROW], F16, name=f"row{tag}_{t}",
                              tag="row")
                nc.scalar.activation(out=row[:, : dout + 1],
                                     in_=mp[:, : dout + 1],
                                     func=mybir.ActivationFunctionType.Copy)
                nc.vector.tensor_scalar(
                    out=adst_sb[:, t:t + 1], in0=mp[:, dout + 1: dout + 2],
                    scalar1=bcol, scalar2=None, op0=mybir.AluOpType.add)
                if t < TA:
                    nc.sync.dma_start(out=dest_a[t * P:(t + 1) * P, :],
                                      in_=row[:])
                else:
                    nc.sync.dma_start(
                        out=dest_b[(t - TA) * P:(t - TA + 1) * P, :],
                        in_=row[:])

            identf = cp.tile([P, P], F32, name="identf")
            make_identity(nc, identf[:])

            def seg_sum(t, bufs, df, tag):
                """psum tile with the chunk-sum for tile t (or None)."""
                nch = int(CA[t] + CB[t])
                if nch == 0:
                    return None
                ps = fpp.tile([P, df], F32, name=f"ps{tag}_{t}", tag="fps")
                k = 0
                for (bk, s0, n) in chunk_groups(t):
                    b = bufs[bk]
                    for si in range(s0, s0 + n):
                        nc.tensor.matmul(
                            out=ps[:], lhsT=ident[:],
                            rhs=b[:, si * ROW: si * ROW + df],
                            start=(k == 0), stop=(k == nch - 1))
                        k += 1
                return ps

            for rep in range(repeat):
                htabs = []
                for pnum in range(2):
                    htabs.append((
                        dp.tile([rowsA, ROW], F16, name=f"htA{pnum}_{rep}",
                                addr_space="Shared"),
                        dp.tile([rowsB, ROW], F16, name=f"htB{pnum}_{rep}",
                                addr_space="Shared")))
                t1tabA = dp.tile([rowsA, ROW], F16, name=f"t1A_{rep}",
                                 addr_space="Shared")
                t1tabB = dp.tile([rowsB, ROW], F16, name=f"t1B_{rep}",
                                 addr_space="Shared")
                t2tabA = dp.tile([rowsA, ROW], F16, name=f"t2A_{rep}",
                                 addr_space="Shared")
                t2tabB = dp.tile([rowsB, ROW], F16, name=f"t2B_{rep}",
                                 addr_space="Shared")
                hins = []
                for pnum in range(2):
                    hins.append((
                        dp.tile([TA * P, ROW], F16, name=f"hiA{pnum}_{rep}"),
                        dp.tile([TB * P, ROW], F16, name=f"hiB{pnum}_{rep}")))
                t1inA = dp.tile([TA * P, ROW], F16, name=f"t1iA_{rep}")
                t1inB = dp.tile([TB * P, ROW], F16, name=f"t1iB_{rep}")
                t2inA = dp.tile([TA * P, ROW], F16, name=f"t2iA_{rep}")
                t2inB = dp.tile([TB * P, ROW], F16, name=f"t2iB_{rep}")

                nc.sync.dma_start(out=acc[:], in_=x_shard_t[:])

                def smoothing_pass(tabA_ap, tabB_ap, pnum, rep=rep):
                    bufs = emit_gathers(tabA_ap, tabB_ap, f"s{pnum}r{rep}")
                    need_ag = pnum < 2
                    if need_ag:
                        hinA, hinB = hins[pnum]
                        htabA, htabB = htabs[pnum]
                    for t in range(TPC):
                        ps = seg_sum(t, bufs, D, f"s{pnum}r{rep}")
                        h = sp.tile([P, D], F32, name=f"h{pnum}_{t}_{rep}",
                                    tag="h")
                        if ps is None:
                            nc.vector.memset(h[:], 0.0)
                        else:
                            nc.vector.tensor_scalar(
                                out=h[:], in0=ps[:],
                                scalar1=rdeg[:, t:t + 1],
                                scalar2=None, op0=mybir.AluOpType.mult)
                        nc.vector.tensor_tensor(
                            out=acc[:, t * D:(t + 1) * D],
                            in0=acc[:, t * D:(t + 1) * D], in1=h[:],
                            op=mybir.AluOpType.add)
                        if need_ag:
                            hrow = wp.tile([P, ROW], F16,
                                           name=f"hr{pnum}_{t}_{rep}",
                                           tag="row")
                            nc.scalar.activation(
                                out=hrow[:, :D], in_=h[:],
                                func=mybir.ActivationFunctionType.Copy)
                            if t < TA:
                                nc.sync.dma_start(
                                    out=hinA[t * P:(t + 1) * P, :],
                                    in_=hrow[:])
                                if t == TA - 1:
                                    nc.gpsimd.collective_compute(
                                        "AllGather", mybir.AluOpType.bypass,
                                        ins=[hinA.opt()], outs=[htabA.opt()],
                                        replica_groups=RG)
                            else:
                                nc.sync.dma_start(
                                    out=hinB[(t - TA) * P:(t - TA + 1) * P,
                                             :],
                                    in_=hrow[:])
                    if need_ag:
                        nc.gpsimd.collective_compute(
                            "AllGather", mybir.AluOpType.bypass,
                            ins=[hinB.opt()], outs=[htabB.opt()],
                            replica_groups=RG)

                smoothing_pass(xtabA_in.ap(), xtabB_in.ap(), 0)
                smoothing_pass(htabs[0][0][:], htabs[0][1][:], 1)

                # pass s3 fused with conv1 projection + sliced t1 AllGather
                bufs3 = emit_gathers(htabs[1][0][:], htabs[1][1][:],
                                     f"s2r{rep}")
                for t in range(TPC):
                    ps = seg_sum(t, bufs3, D, f"s2r{rep}")
                    h = sp.tile([P, D], F32, name=f"h2_{t}_{rep}", tag="h")
                    if ps is None:
                        nc.vector.memset(h[:], 0.0)
                    else:
                        nc.vector.tensor_scalar(
                            out=h[:], in0=ps[:], scalar1=rdeg[:, t:t + 1],
                            scalar2=None, op0=mybir.AluOpType.mult)
                    nc.vector.tensor_tensor(
                        out=acc[:, t * D:(t + 1) * D],
                        in0=acc[:, t * D:(t + 1) * D], in1=h[:],
                        op=mybir.AluOpType.add)
                    proj_tile(t, acc[:, t * D:(t + 1) * D], wcat1, D, D,
                              t1inA, t1inB, adst1, params[:, 0:1],
                              f"t1_{rep}")
                    if t == TA - 1:
                        nc.gpsimd.collective_compute(
                            "AllGather", mybir.AluOpType.bypass,
                            ins=[t1inA.opt()], outs=[t1tabA.opt()],
                            replica_groups=RG)
                nc.gpsimd.collective_compute(
                    "AllGather", mybir.AluOpType.bypass,
                    ins=[t1inB.opt()], outs=[t1tabB.opt()],
                    replica_groups=RG)

                def conv_pass(tabA_ap, tabB_ap, df, adst_sb, pnum, post_fn,
                              rep=rep):
                    bufs = emit_gathers(tabA_ap, tabB_ap, f"c{pnum}r{rep}")
                    for t in range(TPC):
                        nch = int(CA[t] + CB[t])
                        if nch == 0:
                            post_fn(t, None)
                            continue
                        ps = fpp.tile([P, df], F32, name=f"cp{pnum}_{t}_{rep}",
                                      tag="fps")
                        k = 0
                        for (bk, s0, n) in chunk_groups(t):
                            b = bufs[bk]
                            g3 = b[:, s0 * ROW:(s0 + n) * ROW].rearrange(
                                "p (c d) -> p c d", d=ROW)
                            z = sp.tile([P, WIN], F16,
                                        name=f"z{pnum}_{t}_{k}_{rep}", tag="z")
                            nc.vector.tensor_scalar(
                                out=z[:, :n].rearrange(
                                    "p (c u) -> p c u", u=1),
                                in0=g3[:, :, df:df + 1],
                                scalar1=adst_sb[:, t:t + 1], scalar2=None,
                                op0=mybir.AluOpType.add)
                            sc = sp.tile([P, WIN], F16,
                                         name=f"sc{pnum}_{t}_{k}_{rep}",
                                         tag="sc")
                            nc.scalar.activation(
                                out=sc[:, :n], in_=z[:, :n],
                                func=mybir.ActivationFunctionType.Lrelu,
                                alpha=NEG_SLOPE)
                            w8 = wp.tile([P, WIN * D], F16,
                                         name=f"w8{pnum}_{t}_{k}_{rep}",
                                         tag="w8")
                            nc.vector.tensor_tensor(
                                out=w8[:, : n * df].rearrange(
                                    "p (c d) -> p c d", d=df),
                                in0=g3[:, :, 0:df],
                                in1=sc[:, :n].to_broadcast([P, n, df]),
                                op=mybir.AluOpType.mult)
                            for si in range(n):
                                nc.tensor.matmul(
                                    out=ps[:], lhsT=ident[:],
                                    rhs=w8[:, si * df:(si + 1) * df],
                                    start=(k == 0), stop=(k == nch - 1))
                                k += 1
                        post_fn(t, ps)

                def post1(t, ps, rep=rep):
                    h1 = sp.tile([P, DH], F32, name=f"h1_{t}_{rep}", tag="h1")
                    if ps is None:
                        nc.vector.memset(h1[:], 0.0)
                    else:
                        nc.scalar.activation(
                            out=h1[:], in_=ps[:],
                            func=mybir.ActivationFunctionType.Relu)
                    proj_tile(t, h1[:], wcat2, DH, DO, t2inA, t2inB, adst2,
                              params[:, 1:2], f"t2_{rep}")
                    if t == TA - 1:
                        nc.gpsimd.collective_compute(
                            "AllGather", mybir.AluOpType.bypass,
                            ins=[t2inA.opt()], outs=[t2tabA.opt()],
                            replica_groups=RG)

                conv_pass(t1tabA[:], t1tabB[:], D, adst1, 1, post1)
                nc.gpsimd.collective_compute(
                    "AllGather", mybir.AluOpType.bypass,
                    ins=[t2inB.opt()], outs=[t2tabB.opt()],
                    replica_groups=RG)

                def post2(t, ps, rep=rep):
                    o = sp.tile([P, DO], F32, name=f"o_{t}_{rep}", tag="o")
                    if ps is None:
                        nc.vector.memset(o[:], 0.0)
                    else:
                        nc.scalar.activation(
                            out=o[:], in_=ps[:],
                            func=mybir.ActivationFunctionType.Copy)
                    nc.sync.dma_start(out=out_sh[t * P:(t + 1) * P, :],
                                      in_=o[:])

                conv_pass(t2tabA[:], t2tabB[:], DO, adst2, 2, post2)

    nc.compile()
    return nc


# ---------------------------------------------------------------- driver

_CACHE = {}


def _get_runner(s, D, DH, DO, repeat):
    key = (s["NP"], s["TOTC"], s["TOT16"], tuple(int(v) for v in s["CA"]),
           tuple(int(v) for v in s["CB"]), D, DH, DO, repeat)
    if key not in _CACHE:
        nc = _build_program(s, D, DH, DO, repeat)
        _CACHE[key] = _Runner(nc, M)
    return _CACHE[key]


def _prep_inputs(s, x, W_att1, b_att1, W_lin1, W_att2, b_att2, W_lin2):
    NP, TPC = s["NP"], s["TPC"]
    N, D = x.shape
    DH = W_lin1.shape[1]
    DO = W_lin2.shape[1]
    pi, tab_row, is_A = s["pi"], s["tab_row"], s["is_A"]
    rowsA, rowsB = s["rowsA"], s["rowsB"]

    real = np.arange(N)
    xtabA = np.zeros((rowsA, ROW), np.float16)
    xtabB = np.zeros((rowsB, ROW), np.float16)
    ra = real[is_A[real]]
    rb = real[~is_A[real]]
    xtabA[tab_row[ra], :D] = x[ra].astype(np.float16)
    xtabB[tab_row[rb], :D] = x[rb].astype(np.float16)

    x_full = np.zeros((NP, D), np.float32)
    x_full[pi[:N]] = x
    x_sh = x_full.reshape(M, TPC, P, D)

    wcat1 = np.concatenate(
        [W_lin1, W_att1[:D, :1], W_att1[D:, :1]], axis=1) * 0.25
    wcat2 = np.concatenate(
        [W_lin2, W_att2[:DH, :1], W_att2[DH:, :1]], axis=1)
    params = np.zeros((P, 2), np.float32)
    params[:, 0] = float(np.asarray(b_att1).reshape(-1)[0])
    params[:, 1] = float(np.asarray(b_att2).reshape(-1)[0])

    in_maps = []
    for c in range(M):
        in_maps.append({
            "xtabA_in": xtabA,
            "xtabB_in": xtabB,
            "x_shard_t": np.ascontiguousarray(
                x_sh[c].transpose(1, 0, 2)).reshape(P, TPC * D),
            "idx_in": s["idx_res"][c],
            "rdeg_in": s["rdeg_ct"][c],
            "wcat1_in": wcat1.astype(np.float32),
            "wcat2_in": wcat2.astype(np.float32),
            "params_in": params,
        })
    return in_maps


def kernel(x, edge_index, W_att1, b_att1, W_lin1, W_att2, b_att2, W_lin2):
    x = np.asarray(x, np.float32)
    edge_index = np.asarray(edge_index)
    N, D = x.shape
    W_lin1 = np.asarray(W_lin1, np.float32)
    W_lin2 = np.asarray(W_lin2, np.float32)
    DH = W_lin1.shape[1]
    DO = W_lin2.shape[1]
    src = edge_index[0].astype(np.int64)
    dst = edge_index[1].astype(np.int64)

    s = _build_schedule(src, dst, N)
    repeat = int(os.environ.get("CC_GCN_REPEAT", "1"))
    r = _get_runner(s, D, DH, DO, repeat)
    in_maps = _prep_inputs(s, x, np.asarray(W_att1, np.float32),
                           np.asarray(b_att1, np.float32), W_lin1,
                           np.asarray(W_att2, np.float32),
                           np.asarray(b_att2, np.float32), W_lin2)
    res = r.run(in_maps)

    pi = s["pi"]
    out_pi = np.concatenate([res[c]["out_sh"] for c in range(M)], axis=0)
    return np.ascontiguousarray(out_pi[pi[:N]]).astype(np.float32)
